# revision 16
# baseline (speedup 1.0000x reference)
"""Trainium2 Bass kernel for nn_EquivariantLocalScoreMachine.

Math: for each query pixel q (B*H*W=4096, 27-dim circular 3x3 patch xq) over
dataset patches p (N*H*W=32768, zero-padded 3x3 patches):
    log_w[q,p] = -(||xq - mu*patch_p||^2) / (2 sigma^2)
               = b[q] + a*<xq, patch_p> + c[p],   a = mu/sigma^2
The per-q term b[q] cancels in the final ratio and is dropped.  Output:
    out[q,ch] = (mu * wc[q,ch]/sum_w[q] - x[q,ch]) / sigma^2
with softmax-style weights over p.

Sharding: patch axis split across 8 cores (4096 each); each core computes
partial (sum_w, wc) for all queries under a per-q shift; host merges with an
exact logsumexp rescale in fp64.

v2 design: the score matmul directly produces u = A'*(g - s) + B'' in PSUM,
where A' = 128*log2(e) and B'' = 16256 - 5.513.  In this scale, u IS the
bf16 bit pattern of ~exp(g-s) (Schraudolph).  The exp work is then split
across two engines:
  - ScalarE chunks: activation Exp with free affine scale=1/A', bias=-B''/A'
    (exact exp, bf16 out).
  - DVE chunks: one stock tensor_scalar int16(max(u,0)) whose int16 output
    bits, reinterpreted as bf16, are exp(g-s)*(1 +- 3%).  z < -88 clamps to
    +0.0; round-to-nearest int convert verified on HW.
Combined exp throughput 1.2+0.96 = 2.16 G elem/s/lane vs 1.2 ScalarE-only.

The weighted-center accumulation uses 4 concurrent col-group matmuls
(tile_position=(0,32j), M=8 each) so 4 p-chunks accumulate per 512-col
stream; partial groups are summed on the host.

Per core: subset pass (stride-16 patch subset, bf16 matmul + DVE reduce_max)
estimates the per-q max; the shift row v = -A'*(g_sub + MARGIN) + B'' is
transposed into xa row 0 via a -A'-scaled identity matmul and exported so the
host knows the exact shift s_eff = (B'' - v)/A'.

Every TPB instruction in this walrus build may carry at most ONE sync wait:
tiny PE "fence" matmuls pre-absorb cross-engine semaphores on hot paths, and
a post-scheduling pass splits any remaining multi-wait instruction into
single-wait NoOps.
"""
import sys
import numpy as np

for _p in ("/opt/trn_rl_repo", "/opt/pypackages"):
    if _p not in sys.path:
        sys.path.append(_p)

import ml_dtypes

BF16 = ml_dtypes.bfloat16

B, C, H, W = 4, 3, 32, 32
N_IMG = 32
NQ = B * H * W            # 4096 queries
NP = N_IMG * H * W        # 32768 dataset patches
NCORES = 8
PLOC = NP // NCORES       # 4096 patches per core
NCHUNK = PLOC // 128      # 32 p-chunks per core
NQC = NQ // 128           # 32 q-chunks (subset pass)
NT = NQ // 512            # 8 q-tiles (main pass)
FD = 1024                 # A-tile free dim (2 chunks per exp call)
SUB_STRIDE = 16
NSUB = PLOC // SUB_STRIDE  # 256 subset patches per core (max gap 95 < 128)
MARGIN = 40.0
KA = 111                  # packed contraction: 4*27 + c_hi + c_lo + shift

AP_SCALE = np.float32(184.66266)      # A' = 128*log2(e)
BPP = np.float32(16256.0 - 5.513)     # B'': bf16 bias minus Schraudolph center
K0 = np.float32(BPP - AP_SCALE * np.float32(MARGIN))
EXP_SCALE = float(1.0 / np.float64(AP_SCALE))
EXP_BIAS = float(-np.float64(BPP) / np.float64(AP_SCALE))
# pr-groups (2 chunks each) handled by the DVE Schraudolph path; rest ScalarE
DVE_GROUPS = frozenset((1, 3, 5, 7, 9, 11, 13))

_prog_cache = {}


def _build_program():
    if "nc" in _prog_cache:
        return _prog_cache["nc"]
    from contextlib import ExitStack
    import concourse.bass as bass
    import concourse.tile as tile
    from concourse import mybir

    f32 = mybir.dt.float32
    bf = mybir.dt.bfloat16
    i16 = mybir.dt.int16
    nc = bass.Bass("TRN2", num_devices=NCORES, debug=False)
    patm_d = nc.dram_tensor("patm", [KA, PLOC], bf, kind="ExternalInput").ap()
    xa_d = nc.dram_tensor("xa", [KA, NQ], bf, kind="ExternalInput").ap()
    xs_d = nc.dram_tensor("xs", [32, NQ], bf, kind="ExternalInput").ap()
    subp_d = nc.dram_tensor("subp", [32, NSUB], bf, kind="ExternalInput").ap()
    pw_d = nc.dram_tensor("pw", [128, 256], bf, kind="ExternalInput").ap()
    ident_d = nc.dram_tensor("ident", [128, 128], bf, kind="ExternalInput").ap()
    out_d = nc.dram_tensor("out", [32, NQ], f32, kind="ExternalOutput").ap()
    srow_d = nc.dram_tensor("srow", [1, NQ], bf, kind="ExternalOutput").ap()

    with tile.TileContext(nc) as tc, ExitStack() as ctx:
        consts = ctx.enter_context(tc.tile_pool(name="consts", bufs=1))
        # PSUM (8 banks): psA 3x[128,1024]=6, scratch bank 1 (subset scores +
        # fence junk + transposed shift row), acc 1x[128,512]=1
        ps_big = ctx.enter_context(tc.tile_pool(name="ps_big", bufs=3, space="PSUM"))
        ps_fence = ctx.enter_context(
            tc.tile_pool(name="ps_fence", bufs=1, space="PSUM"))
        ps_acc = ctx.enter_context(tc.tile_pool(name="ps_acc", bufs=1, space="PSUM"))
        wpool = ctx.enter_context(tc.tile_pool(name="wpool", bufs=6))
        spool = ctx.enter_context(tc.tile_pool(name="spool", bufs=2))
        dpool = ctx.enter_context(tc.tile_pool(name="dpool", bufs=2))

        # All sync DMAs serialize on one HWDGE queue (~100GB/s), so order by
        # first use. xa row 0 is memset on-device (the shift writes replace
        # it) so the shift path never waits on the 0.9MB bulk transfer; patm
        # is split so the first p-chunks arrive before the main loop needs
        # them.
        s_subp = consts.tile([32, NSUB], bf)
        nc.sync.dma_start(out=s_subp, in_=subp_d)
        s_xs = consts.tile([32, NQ], bf)
        nc.sync.dma_start(out=s_xs, in_=xs_d)
        s_id = consts.tile([128, 128], bf)
        nc.sync.dma_start(out=s_id, in_=ident_d)
        # xa row 0 is NOT transferred or memset: every column is written by a
        # subset-chunk shift add before any reader touches it.
        s_xa = consts.tile([KA, NQ], bf)
        nc.sync.dma_start(out=s_xa[1:KA, 0:512], in_=xa_d[1:KA, 0:512])
        s_patm = consts.tile([KA, PLOC], bf)
        nc.sync.dma_start(out=s_patm[:, 0:1024], in_=patm_d[:, 0:1024])
        s_pw = consts.tile([128, 256], bf)
        nc.scalar.dma_start(out=s_pw, in_=pw_d)
        nc.scalar.dma_start(out=s_xa[1:KA, 512:], in_=xa_d[1:KA, 512:])
        nc.sync.dma_start(out=s_patm[:, 1024:2048], in_=patm_d[:, 1024:2048])
        nc.sync.dma_start(out=s_patm[:, 2048:3072], in_=patm_d[:, 2048:3072])
        nc.sync.dma_start(out=s_patm[:, 3072:], in_=patm_d[:, 3072:])

        # per-partition bias column for the ScalarE exp affine
        s_bias = consts.tile([128, 1], f32)
        nc.vector.memset(s_bias, EXP_BIAS)

        # wfence: long-lived scratch bank. Fence matmuls write junk columns
        # [0:16); [128:256) holds each q-chunk's transposed shift row briefly;
        # [256:256+NSUB) is the subset score region.
        wfence = ps_fence.tile([128, 512], f32, name="wfence")

        def warm_fence(wi, warm):
            nc.tensor.matmul(wfence[0:1, wi:wi + 1], warm[0:32, 0:1],
                             warm[0:32, 0:1], start=True, stop=True)

        # Subset-max work for one q-chunk: bf16 scores over the patch subset,
        # DVE reduce_max, then v = -A'*gsub + K0 transposed into xa row 0.
        gsubs = {}

        def subset_chunk(qc, prologue=False):
            if qc >= 1:
                # absorb the DVE tick of reduce(qc-1), whose scratch region
                # (or pool slot) the matmuls below overwrite
                nc.tensor.matmul(wfence[0:1, 8 + (qc % 2):9 + (qc % 2)],
                                 gsubs[qc - 1], s_id[0:128, 0:1],
                                 start=True, stop=True)
            if prologue:
                ps_sub = ps_big.tile([128, NSUB], f32, tag="big",
                                     name="ps_sub")
            else:
                ps_sub = wfence[:, 256:256 + NSUB]
            for h in range((NSUB + 511) // 512):
                n0, n1 = h * 512, min((h + 1) * 512, NSUB)
                nc.tensor.matmul(
                    ps_sub[:, n0:n1],
                    s_xs[:, qc * 128:(qc + 1) * 128],
                    s_subp[:, n0:n1],
                    start=True, stop=True,
                )
            # bf16 gsub (error < +-2, absorbed by MARGIN; exact shift is
            # exported via srow so no output error)
            gsub = spool.tile([128, 1], bf, name="gsub")
            gsubs[qc] = gsub
            nc.vector.reduce_max(out=gsub, in_=ps_sub, axis=mybir.AxisListType.X)
            # rowp = -A' * gsub  (s_id = -A'*I)
            rowp = wfence[0:1, 128:256]
            nc.tensor.matmul(rowp, gsub, s_id, start=True, stop=True)
            # xa row 0 <- rowp + K0, converted to bf16 on write
            nc.vector.tensor_scalar_add(
                s_xa[0:1, qc * 128:(qc + 1) * 128], rowp, float(K0))

        # Main work for one q-tile: u-scores -> exp split across ScalarE
        # (exact, affine-folded) and DVE (Schraudolph int16 bit trick) ->
        # [wc_hi|wc_lo|sum_w] accumulated in 4 concurrent PSUM col-groups.
        def main_tile(t, weave):
            # absorb the DVE ticks of this q-tile's row-0 shift writes
            nc.tensor.matmul(wfence[0:1, 0:512],
                             s_xa[0:1, t * 512:t * 512 + 1],
                             s_xa[0:1, t * 512:(t + 1) * 512],
                             start=True, stop=True)
            accT = ps_acc.tile([128, 512], f32, tag="acc", name="accT")
            for pr in range(NCHUNK // 2):
                psA = ps_big.tile([128, FD], f32, tag="big", name="psA")
                for i in range(2):
                    ch = pr * 2 + i
                    nc.tensor.matmul(
                        psA[:, i * 512:(i + 1) * 512],
                        s_patm[:, ch * 128:(ch + 1) * 128],
                        s_xa[:, t * 512:(t + 1) * 512],
                        start=True, stop=True,
                    )
                wgt = wpool.tile([128, FD], bf, name="wgt")
                if pr in DVE_GROUPS:
                    nc.vector.tensor_scalar_max(wgt.bitcast(i16), psA, 0.0)
                else:
                    nc.scalar.activation(wgt, psA,
                                         mybir.ActivationFunctionType.Exp,
                                         bias=s_bias, scale=EXP_SCALE)
                for i in range(2):
                    ch = pr * 2 + i
                    j = ch % 4
                    nc.tensor.matmul(
                        accT[32 * j:32 * j + 8, :],
                        s_pw[:, ch * 8:(ch + 1) * 8],
                        wgt[:, i * 512:(i + 1) * 512],
                        start=(ch < 4), stop=(ch >= NCHUNK - 4),
                        tile_position=(0, 32 * j),
                        skip_group_check=True,
                    )
                if weave and pr in (2, 6, 10, 14):
                    subset_chunk(weave.pop(0))
            dr = dpool.tile([128, 512], f32, name="dr")
            nc.vector.tensor_copy(dr, accT)
            for j in range(4):
                nc.sync.dma_start(
                    out=out_d[8 * j:8 * j + 8, t * 512:(t + 1) * 512],
                    in_=dr[32 * j:32 * j + 8, :])

        for wi, warm in enumerate((s_subp, s_xs, s_id)):
            warm_fence(wi, warm)
        for qc in range(4):
            subset_chunk(qc, prologue=True)
        for wi, warm in enumerate((s_xa, s_patm, s_pw)):
            warm_fence(3 + wi, warm)
        for t in range(NT):
            weave = list(range(4 * t + 4, 4 * t + 8)) if t < NT - 1 else []
            main_tile(t, weave)
            if t == NT - 2:
                # row 0 is fully written once tile 6's weave ran; export it
                # while tile 7 computes so it is off the tail
                nc.sync.dma_start(out=srow_d, in_=s_xa[0:1, :])

    # This walrus build permits exactly ONE sync wait per instruction
    # (setupSyncWait raises "Too many sync wait commands" at 2). Tile emits
    # several on converging dependencies; move extras onto preceding
    # same-engine NoOps, each carrying a single wait.
    for blk in nc.m.functions[0].blocks:
        out, changed = [], False
        for ins in blk.instructions:
            si = ins.sync_info
            w = list(si.on_wait) if si is not None and si.on_wait else []
            if len(w) > 1:
                for k, extra in enumerate(w[:-1]):
                    out.append(mybir.InstNoOp(
                        name=f"{ins.name}_sw{k}", engine=ins.engine,
                        sync_info=mybir.SyncInfo(on_wait=[extra], on_update=[]),
                        bass_nofuse=True))
                ins.sync_info = mybir.SyncInfo(
                    on_wait=[w[-1]], on_update=list(si.on_update))
                changed = True
            out.append(ins)
        if changed:
            blk.instructions = out

    _prog_cache["nc"] = nc
    return nc


def _im2col(arr, wrap):
    # [M,C,H,W] -> [M*H*W, C*9]; row m*H*W + h*W + w; col c*9 + di*3 + dj
    if wrap:
        ap = np.pad(arr, ((0, 0), (0, 0), (1, 1), (1, 1)), mode="wrap")
    else:
        ap = np.pad(arr, ((0, 0), (0, 0), (1, 1), (1, 1)))
    sl = [ap[:, :, di:di + H, dj:dj + W] for di in range(3) for dj in range(3)]
    st = np.stack(sl, axis=2)  # [M,C,9,H,W]
    return st.transpose(0, 3, 4, 1, 2).reshape(arr.shape[0] * H * W, C * 9)


def _split(v):
    # fp32 -> (hi, lo) bf16 pair with v ~= hi + lo to ~2^-16 relative
    h = v.astype(BF16)
    l = (v - h.astype(np.float32)).astype(BF16)
    return h, l


def _run(inputs, trace=False):
    from concourse.bass_utils import run_bass_kernel_spmd

    x = np.ascontiguousarray(np.asarray(inputs["x"], dtype=np.float32))
    images = np.ascontiguousarray(np.asarray(inputs["images"], dtype=np.float32))
    t = int(np.asarray(inputs["t"]))
    mu = float(np.asarray(inputs["mu_sched"])[t])
    sigma = float(np.asarray(inputs["sigma_sched"])[t])
    a = mu / (sigma * sigma)
    inv2s2 = 1.0 / (2.0 * sigma * sigma)

    xq = _im2col(x, wrap=True)                  # [NQ, 27]
    patches = _im2col(images, wrap=False)       # [NP, 27]
    c_all = ((-mu * mu * inv2s2) * np.sum(patches * patches, axis=1)).astype(np.float32)
    pcent = patches.reshape(NP, C, 9)[:, :, 4]  # [NP, 3]

    # x-side rows pre-scaled by A' so PSUM = A'*(g - s) + B'' directly
    xh, xl = _split((AP_SCALE * np.float32(a) * xq).astype(np.float32))
    xhu, _ = _split((a * xq).astype(np.float32))   # unscaled, subset pass

    xa = np.zeros((KA, NQ), BF16)
    xa[1:28] = xh.T
    xa[28:55] = xh.T
    xa[55:82] = xl.T
    xa[82:109] = xl.T
    xa[109] = BF16(1.0)
    xa[110] = BF16(1.0)

    xs = np.zeros((32, NQ), BF16)               # subset-pass lhsT (hi only)
    xs[0:27] = xhu.T
    xs[27] = BF16(1.0)

    # -A' * identity: the shift transpose directly scales gsub
    ident = (-float(AP_SCALE) * np.eye(128, dtype=np.float32)).astype(BF16)

    in_maps = []
    for cc in range(NCORES):
        lo = cc * PLOC
        ph, pl = _split(patches[lo:lo + PLOC])
        ch, cl = _split((AP_SCALE * c_all[lo:lo + PLOC]).astype(np.float32))
        patm = np.zeros((KA, PLOC), BF16)
        patm[0] = BF16(1.0)
        patm[1:28] = ph.T
        patm[28:55] = pl.T
        patm[55:82] = ph.T
        patm[82:109] = pl.T
        patm[109] = ch
        patm[110] = cl
        subp = np.zeros((32, NSUB), BF16)
        subp[0:27] = ph.T[:, ::SUB_STRIDE]
        subp[27] = c_all[lo:lo + PLOC][::SUB_STRIDE].astype(BF16)
        pwh, pwl = _split(pcent[lo:lo + PLOC])
        pw = np.zeros((128, 256), BF16)
        for chnk in range(NCHUNK):
            pw[:, chnk * 8:chnk * 8 + 3] = pwh[chnk * 128:(chnk + 1) * 128]
            pw[:, chnk * 8 + 3:chnk * 8 + 6] = pwl[chnk * 128:(chnk + 1) * 128]
            pw[:, chnk * 8 + 6] = BF16(1.0)
        in_maps.append({
            "patm": patm, "xa": xa, "xs": xs, "subp": subp, "pw": pw,
            "ident": ident,
        })

    nc = _build_program()
    res = run_bass_kernel_spmd(nc, in_maps, core_ids=list(range(NCORES)),
                               trace=trace)

    # host merge: partials are scaled by e^{-s_eff_c}; rescale to common max
    # s_eff = (B'' - v)/A' recovered exactly from the exported bf16 row
    v = np.stack([r["srow"][0].astype(np.float64) for r in res.results])
    s = (np.float64(BPP) - v) / np.float64(AP_SCALE)          # [8, NQ]
    part = np.stack([r["out"] for r in res.results])          # [8, 32, NQ]
    part = part.reshape(NCORES, 4, 8, NQ).sum(axis=1)         # sum col-groups
    S = s.max(axis=0)
    fac = np.exp(s - S[None, :])                              # [8, NQ] <= 1
    sum_w = (part[:, 6].astype(np.float64) * fac).sum(axis=0)              # [NQ]
    wc = ((part[:, 0:3] + part[:, 3:6]).astype(np.float64)
          * fac[:, None, :]).sum(axis=0)                      # [3, NQ]

    xcT = x.reshape(B, C, H * W).transpose(1, 0, 2).reshape(C, NQ)
    out_q = (mu * wc / sum_w[None, :] - xcT) / (sigma * sigma)  # [3, NQ]
    out = out_q.reshape(C, B, H, W).transpose(1, 0, 2, 3).astype(np.float32)
    return out, res


def kernel(**inputs) -> np.ndarray:
    out, _ = _run(inputs, trace=False)
    return out


# revision 17
# speedup vs baseline: 1.2127x; 1.2127x over previous
"""Trainium2 Bass kernel for nn_EquivariantLocalScoreMachine.

Math: for each query pixel q (B*H*W=4096, 27-dim circular 3x3 patch xq) over
dataset patches p (N*H*W=32768, zero-padded 3x3 patches):
    log_w[q,p] = -(||xq - mu*patch_p||^2) / (2 sigma^2)
               = b[q] + a*<xq, patch_p> + c[p],   a = mu/sigma^2
The per-q term b[q] cancels in the final ratio and is dropped.  Output:
    out[q,ch] = (mu * wc[q,ch]/sum_w[q] - x[q,ch]) / sigma^2
with softmax-style weights over p.

Sharding: patch axis split across 8 cores (4096 each); each core computes
partial (sum_w, wc) for all queries under a per-q shift; host merges with an
exact logsumexp rescale in fp64.

v2 design: the score matmul directly produces u = A'*(g - s) + B'' in PSUM,
where A' = 128*log2(e) and B'' = 16256 - 5.513.  In this scale, u IS the
bf16 bit pattern of ~exp(g-s) (Schraudolph).  The exp work is then split
across two engines:
  - ScalarE chunks: activation Exp with free affine scale=1/A', bias=-B''/A'
    (exact exp, bf16 out).
  - DVE chunks: one stock tensor_scalar int16(max(u,0)) whose int16 output
    bits, reinterpreted as bf16, are exp(g-s)*(1 +- 3%).  z < -88 clamps to
    +0.0; round-to-nearest int convert verified on HW.
Combined exp throughput 1.2+0.96 = 2.16 G elem/s/lane vs 1.2 ScalarE-only.

The weighted-center accumulation uses 4 concurrent col-group matmuls
(tile_position=(0,32j), M=8 each) so 4 p-chunks accumulate per 512-col
stream; partial groups are summed on the host.

Per core: subset pass (stride-16 patch subset, bf16 matmul + DVE reduce_max)
estimates the per-q max; the shift row v = -A'*(g_sub + MARGIN) + B'' is
transposed into xa row 0 via a -A'-scaled identity matmul and exported so the
host knows the exact shift s_eff = (B'' - v)/A'.

Every TPB instruction in this walrus build may carry at most ONE sync wait:
tiny PE "fence" matmuls pre-absorb cross-engine semaphores on hot paths, and
a post-scheduling pass splits any remaining multi-wait instruction into
single-wait NoOps.
"""
import sys
import numpy as np

for _p in ("/opt/trn_rl_repo", "/opt/pypackages"):
    if _p not in sys.path:
        sys.path.append(_p)

import ml_dtypes

BF16 = ml_dtypes.bfloat16

B, C, H, W = 4, 3, 32, 32
N_IMG = 32
NQ = B * H * W            # 4096 queries
NP = N_IMG * H * W        # 32768 dataset patches
NCORES = 8
PLOC = NP // NCORES       # 4096 patches per core
NCHUNK = PLOC // 128      # 32 p-chunks per core
NQC = NQ // 128           # 32 q-chunks (subset pass)
NT = NQ // 512            # 8 q-tiles (main pass)
FD = 1024                 # A-tile free dim (2 chunks per exp call)
SUB_STRIDE = 16
NSUB = PLOC // SUB_STRIDE  # 256 subset patches per core (max gap 95 < 128)
MARGIN = 40.0
KA = 111                  # packed contraction: 4*27 + c_hi + c_lo + shift

AP_SCALE = np.float32(184.66266)      # A' = 128*log2(e)
BPP = np.float32(16256.0 - 5.513)     # B'': bf16 bias minus Schraudolph center
K0 = np.float32(BPP - AP_SCALE * np.float32(MARGIN))
EXP_SCALE = float(1.0 / np.float64(AP_SCALE))
EXP_BIAS = float(-np.float64(BPP) / np.float64(AP_SCALE))
# pr-groups (2 chunks each) handled by the DVE Schraudolph path; rest ScalarE
DVE_GROUPS = frozenset((1, 3, 5, 7, 9, 11, 13))

_prog_cache = {}


def _build_program():
    if "nc" in _prog_cache:
        return _prog_cache["nc"]
    from contextlib import ExitStack
    import concourse.bass as bass
    import concourse.tile as tile
    from concourse import mybir

    f32 = mybir.dt.float32
    bf = mybir.dt.bfloat16
    i16 = mybir.dt.int16
    nc = bass.Bass("TRN2", num_devices=NCORES, debug=False)
    patm_d = nc.dram_tensor("patm", [KA, PLOC], bf, kind="ExternalInput").ap()
    xa_d = nc.dram_tensor("xa", [KA, NQ], bf, kind="ExternalInput").ap()
    xs_d = nc.dram_tensor("xs", [32, NQ], bf, kind="ExternalInput").ap()
    subp_d = nc.dram_tensor("subp", [32, NSUB], bf, kind="ExternalInput").ap()
    pw_d = nc.dram_tensor("pw", [128, 256], bf, kind="ExternalInput").ap()
    ident_d = nc.dram_tensor("ident", [128, 128], bf, kind="ExternalInput").ap()
    out_d = nc.dram_tensor("out", [32, NQ], f32, kind="ExternalOutput").ap()
    srow_d = nc.dram_tensor("srow", [1, NQ], bf, kind="ExternalOutput").ap()

    with tile.TileContext(nc) as tc, ExitStack() as ctx:
        consts = ctx.enter_context(tc.tile_pool(name="consts", bufs=1))
        # PSUM (8 banks): psA 3x[128,1024]=6, scratch bank 1 (subset scores +
        # fence junk + transposed shift row), acc 1x[128,512]=1
        ps_big = ctx.enter_context(tc.tile_pool(name="ps_big", bufs=3, space="PSUM"))
        ps_fence = ctx.enter_context(
            tc.tile_pool(name="ps_fence", bufs=1, space="PSUM"))
        ps_acc = ctx.enter_context(tc.tile_pool(name="ps_acc", bufs=1, space="PSUM"))
        wpool = ctx.enter_context(tc.tile_pool(name="wpool", bufs=6))
        spool = ctx.enter_context(tc.tile_pool(name="spool", bufs=2))
        dpool = ctx.enter_context(tc.tile_pool(name="dpool", bufs=2))

        # All sync DMAs serialize on one HWDGE queue (~100GB/s), so order by
        # first use. xa row 0 is memset on-device (the shift writes replace
        # it) so the shift path never waits on the 0.9MB bulk transfer; patm
        # is split so the first p-chunks arrive before the main loop needs
        # them.
        s_subp = consts.tile([32, NSUB], bf)
        nc.sync.dma_start(out=s_subp, in_=subp_d)
        s_xs = consts.tile([32, NQ], bf)
        nc.sync.dma_start(out=s_xs, in_=xs_d)
        s_id = consts.tile([128, 128], bf)
        nc.sync.dma_start(out=s_id, in_=ident_d)
        # xa row 0 is NOT transferred or memset: every column is written by a
        # subset-chunk shift add before any reader touches it.
        s_xa = consts.tile([KA, NQ], bf)
        nc.sync.dma_start(out=s_xa[1:KA, 0:512], in_=xa_d[1:KA, 0:512])
        s_patm = consts.tile([KA, PLOC], bf)
        nc.sync.dma_start(out=s_patm[:, 0:1024], in_=patm_d[:, 0:1024])
        s_pw = consts.tile([128, 256], bf)
        nc.sync.dma_start(out=s_pw, in_=pw_d)
        nc.sync.dma_start(out=s_patm[:, 1024:2048], in_=patm_d[:, 1024:2048])
        nc.sync.dma_start(out=s_patm[:, 2048:3072], in_=patm_d[:, 2048:3072])
        nc.sync.dma_start(out=s_patm[:, 3072:], in_=patm_d[:, 3072:])
        nc.sync.dma_start(out=s_xa[1:KA, 512:], in_=xa_d[1:KA, 512:])

        # per-partition bias column for the ScalarE exp affine
        s_bias = consts.tile([128, 1], f32)
        nc.vector.memset(s_bias, EXP_BIAS)

        # wfence: long-lived scratch bank. Fence matmuls write junk columns
        # [0:16); [128:256) holds each q-chunk's transposed shift row briefly;
        # [256:256+NSUB) is the subset score region.
        wfence = ps_fence.tile([128, 512], f32, name="wfence")

        def warm_fence(wi, warm):
            nc.tensor.matmul(wfence[0:1, wi:wi + 1], warm[0:32, 0:1],
                             warm[0:32, 0:1], start=True, stop=True)

        # Subset-max work for one q-chunk: bf16 scores over the patch subset,
        # DVE reduce_max, then v = -A'*gsub + K0 transposed into xa row 0.
        gsubs = {}

        def subset_chunk(qc, prologue=False):
            if qc >= 1:
                # absorb the DVE tick of reduce(qc-1), whose scratch region
                # (or pool slot) the matmuls below overwrite
                nc.tensor.matmul(wfence[0:1, 8 + (qc % 2):9 + (qc % 2)],
                                 gsubs[qc - 1], s_id[0:128, 0:1],
                                 start=True, stop=True)
            if prologue:
                ps_sub = ps_big.tile([128, NSUB], f32, tag="big",
                                     name="ps_sub")
            else:
                ps_sub = wfence[:, 256:256 + NSUB]
            for h in range((NSUB + 511) // 512):
                n0, n1 = h * 512, min((h + 1) * 512, NSUB)
                nc.tensor.matmul(
                    ps_sub[:, n0:n1],
                    s_xs[:, qc * 128:(qc + 1) * 128],
                    s_subp[:, n0:n1],
                    start=True, stop=True,
                )
            # bf16 gsub (error < +-2, absorbed by MARGIN; exact shift is
            # exported via srow so no output error)
            gsub = spool.tile([128, 1], bf, name="gsub")
            gsubs[qc] = gsub
            nc.vector.reduce_max(out=gsub, in_=ps_sub, axis=mybir.AxisListType.X)
            # rowp = -A' * gsub  (s_id = -A'*I)
            rowp = wfence[0:1, 128:256]
            nc.tensor.matmul(rowp, gsub, s_id, start=True, stop=True)
            # xa row 0 <- rowp + K0, converted to bf16 on write
            nc.vector.tensor_scalar_add(
                s_xa[0:1, qc * 128:(qc + 1) * 128], rowp, float(K0))

        # Main work for one q-tile: u-scores -> exp split across ScalarE
        # (exact, affine-folded) and DVE (Schraudolph int16 bit trick) ->
        # [wc_hi|wc_lo|sum_w] accumulated in 4 concurrent PSUM col-groups.
        def main_tile(t, weave):
            # absorb the DVE ticks of this q-tile's row-0 shift writes
            nc.tensor.matmul(wfence[0:1, 0:512],
                             s_xa[0:1, t * 512:t * 512 + 1],
                             s_xa[0:1, t * 512:(t + 1) * 512],
                             start=True, stop=True)
            accT = ps_acc.tile([128, 512], f32, tag="acc", name="accT")
            for pr in range(NCHUNK // 2):
                psA = ps_big.tile([128, FD], f32, tag="big", name="psA")
                for i in range(2):
                    ch = pr * 2 + i
                    nc.tensor.matmul(
                        psA[:, i * 512:(i + 1) * 512],
                        s_patm[:, ch * 128:(ch + 1) * 128],
                        s_xa[:, t * 512:(t + 1) * 512],
                        start=True, stop=True,
                    )
                wgt = wpool.tile([128, FD], bf, name="wgt")
                if pr in DVE_GROUPS:
                    nc.vector.tensor_scalar_max(wgt.bitcast(i16), psA, 0.0)
                else:
                    nc.scalar.activation(wgt, psA,
                                         mybir.ActivationFunctionType.Exp,
                                         bias=s_bias, scale=EXP_SCALE)
                for i in range(2):
                    ch = pr * 2 + i
                    j = ch % 4
                    nc.tensor.matmul(
                        accT[32 * j:32 * j + 8, :],
                        s_pw[:, ch * 8:(ch + 1) * 8],
                        wgt[:, i * 512:(i + 1) * 512],
                        start=(ch < 4), stop=(ch >= NCHUNK - 4),
                        tile_position=(0, 32 * j),
                        skip_group_check=True,
                    )
                if weave and pr in (2, 6, 10, 14):
                    subset_chunk(weave.pop(0))
            dr = dpool.tile([128, 512], f32, name="dr")
            nc.vector.tensor_copy(dr, accT)
            for j in range(4):
                nc.sync.dma_start(
                    out=out_d[8 * j:8 * j + 8, t * 512:(t + 1) * 512],
                    in_=dr[32 * j:32 * j + 8, :])

        for wi, warm in enumerate((s_subp, s_xs, s_id)):
            warm_fence(wi, warm)
        for qc in range(4):
            subset_chunk(qc, prologue=True)
        for wi, warm in enumerate((s_xa, s_patm, s_pw)):
            warm_fence(3 + wi, warm)
        for t in range(NT):
            weave = list(range(4 * t + 4, 4 * t + 8)) if t < NT - 1 else []
            main_tile(t, weave)
            if t == NT - 2:
                # row 0 is fully written once tile 6's weave ran; export it
                # while tile 7 computes so it is off the tail
                nc.sync.dma_start(out=srow_d, in_=s_xa[0:1, :])

    # This walrus build permits exactly ONE sync wait per instruction
    # (setupSyncWait raises "Too many sync wait commands" at 2). Tile emits
    # several on converging dependencies; move extras onto preceding
    # same-engine NoOps, each carrying a single wait.
    for blk in nc.m.functions[0].blocks:
        out, changed = [], False
        for ins in blk.instructions:
            si = ins.sync_info
            w = list(si.on_wait) if si is not None and si.on_wait else []
            if len(w) > 1:
                for k, extra in enumerate(w[:-1]):
                    out.append(mybir.InstNoOp(
                        name=f"{ins.name}_sw{k}", engine=ins.engine,
                        sync_info=mybir.SyncInfo(on_wait=[extra], on_update=[]),
                        bass_nofuse=True))
                ins.sync_info = mybir.SyncInfo(
                    on_wait=[w[-1]], on_update=list(si.on_update))
                changed = True
            out.append(ins)
        if changed:
            blk.instructions = out

    _prog_cache["nc"] = nc
    return nc


def _im2col(arr, wrap):
    # [M,C,H,W] -> [M*H*W, C*9]; row m*H*W + h*W + w; col c*9 + di*3 + dj
    if wrap:
        ap = np.pad(arr, ((0, 0), (0, 0), (1, 1), (1, 1)), mode="wrap")
    else:
        ap = np.pad(arr, ((0, 0), (0, 0), (1, 1), (1, 1)))
    sl = [ap[:, :, di:di + H, dj:dj + W] for di in range(3) for dj in range(3)]
    st = np.stack(sl, axis=2)  # [M,C,9,H,W]
    return st.transpose(0, 3, 4, 1, 2).reshape(arr.shape[0] * H * W, C * 9)


def _split(v):
    # fp32 -> (hi, lo) bf16 pair with v ~= hi + lo to ~2^-16 relative
    h = v.astype(BF16)
    l = (v - h.astype(np.float32)).astype(BF16)
    return h, l


def _run(inputs, trace=False):
    from concourse.bass_utils import run_bass_kernel_spmd

    x = np.ascontiguousarray(np.asarray(inputs["x"], dtype=np.float32))
    images = np.ascontiguousarray(np.asarray(inputs["images"], dtype=np.float32))
    t = int(np.asarray(inputs["t"]))
    mu = float(np.asarray(inputs["mu_sched"])[t])
    sigma = float(np.asarray(inputs["sigma_sched"])[t])
    a = mu / (sigma * sigma)
    inv2s2 = 1.0 / (2.0 * sigma * sigma)

    xq = _im2col(x, wrap=True)                  # [NQ, 27]
    patches = _im2col(images, wrap=False)       # [NP, 27]
    c_all = ((-mu * mu * inv2s2) * np.sum(patches * patches, axis=1)).astype(np.float32)
    pcent = patches.reshape(NP, C, 9)[:, :, 4]  # [NP, 3]

    # x-side rows pre-scaled by A' so PSUM = A'*(g - s) + B'' directly
    xh, xl = _split((AP_SCALE * np.float32(a) * xq).astype(np.float32))
    xhu, _ = _split((a * xq).astype(np.float32))   # unscaled, subset pass

    xa = np.zeros((KA, NQ), BF16)
    xa[1:28] = xh.T
    xa[28:55] = xh.T
    xa[55:82] = xl.T
    xa[82:109] = xl.T
    xa[109] = BF16(1.0)
    xa[110] = BF16(1.0)

    xs = np.zeros((32, NQ), BF16)               # subset-pass lhsT (hi only)
    xs[0:27] = xhu.T
    xs[27] = BF16(1.0)

    # -A' * identity: the shift transpose directly scales gsub
    ident = (-float(AP_SCALE) * np.eye(128, dtype=np.float32)).astype(BF16)

    in_maps = []
    for cc in range(NCORES):
        lo = cc * PLOC
        ph, pl = _split(patches[lo:lo + PLOC])
        ch, cl = _split((AP_SCALE * c_all[lo:lo + PLOC]).astype(np.float32))
        patm = np.zeros((KA, PLOC), BF16)
        patm[0] = BF16(1.0)
        patm[1:28] = ph.T
        patm[28:55] = pl.T
        patm[55:82] = ph.T
        patm[82:109] = pl.T
        patm[109] = ch
        patm[110] = cl
        subp = np.zeros((32, NSUB), BF16)
        subp[0:27] = ph.T[:, ::SUB_STRIDE]
        subp[27] = c_all[lo:lo + PLOC][::SUB_STRIDE].astype(BF16)
        pwh, pwl = _split(pcent[lo:lo + PLOC])
        pw = np.zeros((128, 256), BF16)
        for chnk in range(NCHUNK):
            pw[:, chnk * 8:chnk * 8 + 3] = pwh[chnk * 128:(chnk + 1) * 128]
            pw[:, chnk * 8 + 3:chnk * 8 + 6] = pwl[chnk * 128:(chnk + 1) * 128]
            pw[:, chnk * 8 + 6] = BF16(1.0)
        in_maps.append({
            "patm": patm, "xa": xa, "xs": xs, "subp": subp, "pw": pw,
            "ident": ident,
        })

    nc = _build_program()
    res = run_bass_kernel_spmd(nc, in_maps, core_ids=list(range(NCORES)),
                               trace=trace)

    # host merge: partials are scaled by e^{-s_eff_c}; rescale to common max
    # s_eff = (B'' - v)/A' recovered exactly from the exported bf16 row
    v = np.stack([r["srow"][0].astype(np.float64) for r in res.results])
    s = (np.float64(BPP) - v) / np.float64(AP_SCALE)          # [8, NQ]
    part = np.stack([r["out"] for r in res.results])          # [8, 32, NQ]
    part = part.reshape(NCORES, 4, 8, NQ).sum(axis=1)         # sum col-groups
    S = s.max(axis=0)
    fac = np.exp(s - S[None, :])                              # [8, NQ] <= 1
    sum_w = (part[:, 6].astype(np.float64) * fac).sum(axis=0)              # [NQ]
    wc = ((part[:, 0:3] + part[:, 3:6]).astype(np.float64)
          * fac[:, None, :]).sum(axis=0)                      # [3, NQ]

    xcT = x.reshape(B, C, H * W).transpose(1, 0, 2).reshape(C, NQ)
    out_q = (mu * wc / sum_w[None, :] - xcT) / (sigma * sigma)  # [3, NQ]
    out = out_q.reshape(C, B, H, W).transpose(1, 0, 2, 3).astype(np.float32)
    return out, res


def kernel(**inputs) -> np.ndarray:
    out, _ = _run(inputs, trace=False)
    return out


# revision 18
# speedup vs baseline: 1.2163x; 1.0030x over previous
"""Trainium2 Bass kernel for nn_EquivariantLocalScoreMachine.

Math: for each query pixel q (B*H*W=4096, 27-dim circular 3x3 patch xq) over
dataset patches p (N*H*W=32768, zero-padded 3x3 patches):
    log_w[q,p] = -(||xq - mu*patch_p||^2) / (2 sigma^2)
               = b[q] + a*<xq, patch_p> + c[p],   a = mu/sigma^2
The per-q term b[q] cancels in the final ratio and is dropped.  Output:
    out[q,ch] = (mu * wc[q,ch]/sum_w[q] - x[q,ch]) / sigma^2
with softmax-style weights over p.

Sharding: patch axis split across 8 cores (4096 each); each core computes
partial (sum_w, wc) for all queries under a per-q shift; host merges with an
exact logsumexp rescale in fp64.

v2 design: the score matmul directly produces u = A'*(g - s) + B'' in PSUM,
where A' = 128*log2(e) and B'' = 16256 - 5.513.  In this scale, u IS the
bf16 bit pattern of ~exp(g-s) (Schraudolph).  The exp work is then split
across two engines:
  - ScalarE chunks: activation Exp with free affine scale=1/A', bias=-B''/A'
    (exact exp, bf16 out).
  - DVE chunks: one stock tensor_scalar int16(max(u,0)) whose int16 output
    bits, reinterpreted as bf16, are exp(g-s)*(1 +- 3%).  z < -88 clamps to
    +0.0; round-to-nearest int convert verified on HW.
Combined exp throughput 1.2+0.96 = 2.16 G elem/s/lane vs 1.2 ScalarE-only.

The weighted-center accumulation uses 4 concurrent col-group matmuls
(tile_position=(0,32j), M=8 each) so 4 p-chunks accumulate per 512-col
stream; partial groups are summed on the host.

Per core: subset pass (stride-16 patch subset, bf16 matmul + DVE reduce_max)
estimates the per-q max; the shift row v = -A'*(g_sub + MARGIN) + B'' is
transposed into xa row 0 via a -A'-scaled identity matmul and exported so the
host knows the exact shift s_eff = (B'' - v)/A'.

Every TPB instruction in this walrus build may carry at most ONE sync wait:
tiny PE "fence" matmuls pre-absorb cross-engine semaphores on hot paths, and
a post-scheduling pass splits any remaining multi-wait instruction into
single-wait NoOps.
"""
import sys
import numpy as np

for _p in ("/opt/trn_rl_repo", "/opt/pypackages"):
    if _p not in sys.path:
        sys.path.append(_p)

import ml_dtypes

BF16 = ml_dtypes.bfloat16

B, C, H, W = 4, 3, 32, 32
N_IMG = 32
NQ = B * H * W            # 4096 queries
NP = N_IMG * H * W        # 32768 dataset patches
NCORES = 8
PLOC = NP // NCORES       # 4096 patches per core
NCHUNK = PLOC // 128      # 32 p-chunks per core
NQC = NQ // 128           # 32 q-chunks (subset pass)
NT = NQ // 512            # 8 q-tiles (main pass)
FD = 1024                 # A-tile free dim (2 chunks per exp call)
SUB_STRIDE = 16
NSUB = PLOC // SUB_STRIDE  # 256 subset patches per core (max gap 95 < 128)
MARGIN = 40.0
KA = 111                  # packed contraction: 4*27 + c_hi + c_lo + shift

AP_SCALE = np.float32(184.66266)      # A' = 128*log2(e)
BPP = np.float32(16256.0 - 5.513)     # B'': bf16 bias minus Schraudolph center
K0 = np.float32(BPP - AP_SCALE * np.float32(MARGIN))
EXP_SCALE = float(1.0 / np.float64(AP_SCALE))
EXP_BIAS = float(-np.float64(BPP) / np.float64(AP_SCALE))
# pr-groups (2 chunks each) handled by the DVE Schraudolph path; rest ScalarE
DVE_GROUPS = frozenset((1, 3, 5, 7, 9, 11, 13))

_prog_cache = {}


def _build_program():
    if "nc" in _prog_cache:
        return _prog_cache["nc"]
    from contextlib import ExitStack
    import concourse.bass as bass
    import concourse.tile as tile
    from concourse import mybir

    f32 = mybir.dt.float32
    bf = mybir.dt.bfloat16
    i16 = mybir.dt.int16
    nc = bass.Bass("TRN2", num_devices=NCORES, debug=False)
    patm_d = nc.dram_tensor("patm", [KA, PLOC], bf, kind="ExternalInput").ap()
    xa_d = nc.dram_tensor("xa", [KA, NQ], bf, kind="ExternalInput").ap()
    xs_d = nc.dram_tensor("xs", [32, NQ], bf, kind="ExternalInput").ap()
    subp_d = nc.dram_tensor("subp", [32, NSUB], bf, kind="ExternalInput").ap()
    pw_d = nc.dram_tensor("pw", [128, 256], bf, kind="ExternalInput").ap()
    ident_d = nc.dram_tensor("ident", [128, 128], bf, kind="ExternalInput").ap()
    out_d = nc.dram_tensor("out", [32, NQ], f32, kind="ExternalOutput").ap()
    srow_d = nc.dram_tensor("srow", [1, NQ], bf, kind="ExternalOutput").ap()

    with tile.TileContext(nc) as tc, ExitStack() as ctx:
        consts = ctx.enter_context(tc.tile_pool(name="consts", bufs=1))
        # PSUM (8 banks): psA 3x[128,1024]=6, scratch bank 1 (subset scores +
        # fence junk + transposed shift row), acc 1x[128,512]=1
        ps_big = ctx.enter_context(tc.tile_pool(name="ps_big", bufs=3, space="PSUM"))
        ps_fence = ctx.enter_context(
            tc.tile_pool(name="ps_fence", bufs=1, space="PSUM"))
        ps_acc = ctx.enter_context(tc.tile_pool(name="ps_acc", bufs=1, space="PSUM"))
        wpool = ctx.enter_context(tc.tile_pool(name="wpool", bufs=6))
        spool = ctx.enter_context(tc.tile_pool(name="spool", bufs=2))
        dpool = ctx.enter_context(tc.tile_pool(name="dpool", bufs=2))

        # All sync DMAs serialize on one HWDGE queue (~100GB/s), so order by
        # first use. xa row 0 is memset on-device (the shift writes replace
        # it) so the shift path never waits on the 0.9MB bulk transfer; patm
        # is split so the first p-chunks arrive before the main loop needs
        # them.
        s_subp = consts.tile([32, NSUB], bf)
        nc.sync.dma_start(out=s_subp, in_=subp_d)
        s_xs = consts.tile([32, NQ], bf)
        nc.sync.dma_start(out=s_xs, in_=xs_d)
        s_id = consts.tile([128, 128], bf)
        nc.sync.dma_start(out=s_id, in_=ident_d)
        # xa row 0 is NOT transferred or memset: every column is written by a
        # subset-chunk shift add before any reader touches it.
        s_xa = consts.tile([KA, NQ], bf)
        nc.sync.dma_start(out=s_xa[1:KA, 0:512], in_=xa_d[1:KA, 0:512])
        s_patm = consts.tile([KA, PLOC], bf)
        nc.sync.dma_start(out=s_patm[:, 0:1024], in_=patm_d[:, 0:1024])
        s_pw = consts.tile([128, 256], bf)
        nc.sync.dma_start(out=s_pw, in_=pw_d)
        nc.sync.dma_start(out=s_patm[:, 1024:2048], in_=patm_d[:, 1024:2048])
        nc.sync.dma_start(out=s_patm[:, 2048:3072], in_=patm_d[:, 2048:3072])
        nc.sync.dma_start(out=s_patm[:, 3072:], in_=patm_d[:, 3072:])
        nc.sync.dma_start(out=s_xa[1:KA, 512:], in_=xa_d[1:KA, 512:])

        # per-partition bias column for the ScalarE exp affine
        s_bias = consts.tile([128, 1], f32)
        nc.vector.memset(s_bias, EXP_BIAS)

        # wfence: long-lived scratch bank. Fence matmuls write junk columns
        # [0:16); [128:256) holds each q-chunk's transposed shift row briefly;
        # [256:256+NSUB) is the subset score region.
        wfence = ps_fence.tile([128, 512], f32, name="wfence")

        def warm_fence(wi, warm):
            nc.tensor.matmul(wfence[0:1, wi:wi + 1], warm[0:32, 0:1],
                             warm[0:32, 0:1], start=True, stop=True)

        # Subset-max work for one q-chunk: bf16 scores over the patch subset,
        # DVE reduce_max, then v = -A'*gsub + K0 transposed into xa row 0.
        gsubs = {}

        def subset_chunk(qc, prologue=False):
            if qc >= 1:
                # absorb the DVE tick of reduce(qc-1), whose scratch region
                # (or pool slot) the matmuls below overwrite
                nc.tensor.matmul(wfence[0:1, 8 + (qc % 2):9 + (qc % 2)],
                                 gsubs[qc - 1], s_id[0:128, 0:1],
                                 start=True, stop=True)
            if prologue:
                ps_sub = ps_big.tile([128, NSUB], f32, tag="big",
                                     name="ps_sub")
            else:
                ps_sub = wfence[:, 256:256 + NSUB]
            for h in range((NSUB + 511) // 512):
                n0, n1 = h * 512, min((h + 1) * 512, NSUB)
                nc.tensor.matmul(
                    ps_sub[:, n0:n1],
                    s_xs[:, qc * 128:(qc + 1) * 128],
                    s_subp[:, n0:n1],
                    start=True, stop=True,
                )
            # bf16 gsub (error < +-2, absorbed by MARGIN; exact shift is
            # exported via srow so no output error)
            gsub = spool.tile([128, 1], bf, name="gsub")
            gsubs[qc] = gsub
            nc.vector.reduce_max(out=gsub, in_=ps_sub, axis=mybir.AxisListType.X)
            # rowp = -A' * gsub  (s_id = -A'*I)
            rowp = wfence[0:1, 128:256]
            nc.tensor.matmul(rowp, gsub, s_id, start=True, stop=True)
            # xa row 0 <- rowp + K0, converted to bf16 on write
            nc.vector.tensor_scalar_add(
                s_xa[0:1, qc * 128:(qc + 1) * 128], rowp, float(K0))

        # Main work for one q-tile: u-scores -> exp split across ScalarE
        # (exact, affine-folded) and DVE (Schraudolph int16 bit trick) ->
        # [wc_hi|wc_lo|sum_w] accumulated in 4 concurrent PSUM col-groups.
        def main_tile(t, weave):
            # absorb the DVE ticks of this q-tile's row-0 shift writes
            nc.tensor.matmul(wfence[0:1, 0:512],
                             s_xa[0:1, t * 512:t * 512 + 1],
                             s_xa[0:1, t * 512:(t + 1) * 512],
                             start=True, stop=True)
            accT = ps_acc.tile([128, 512], f32, tag="acc", name="accT")
            # acc matmuls are deferred one pr-group and emitted in batches of
            # four CONSECUTIVE chunks (4 distinct col-groups): by batch time
            # only the newest wgt still gates, so the PE runs all four as one
            # concurrent col-tiled burst (~1 stream instead of 4).
            pend = []
            for pr in range(NCHUNK // 2):
                psA = ps_big.tile([128, FD], f32, tag="big", name="psA")
                for i in range(2):
                    ch = pr * 2 + i
                    nc.tensor.matmul(
                        psA[:, i * 512:(i + 1) * 512],
                        s_patm[:, ch * 128:(ch + 1) * 128],
                        s_xa[:, t * 512:(t + 1) * 512],
                        start=True, stop=True,
                    )
                wgt = wpool.tile([128, FD], bf, name="wgt")
                if pr in DVE_GROUPS:
                    nc.vector.tensor_scalar_max(wgt.bitcast(i16), psA, 0.0)
                else:
                    nc.scalar.activation(wgt, psA,
                                         mybir.ActivationFunctionType.Exp,
                                         bias=s_bias, scale=EXP_SCALE)
                for i in range(2):
                    pend.append((pr * 2 + i, wgt, i))
                if pr % 2 == 1:
                    for ch, w, i in pend:
                        j = ch % 4
                        nc.tensor.matmul(
                            accT[32 * j:32 * j + 8, :],
                            s_pw[:, ch * 8:(ch + 1) * 8],
                            w[:, i * 512:(i + 1) * 512],
                            start=(ch < 4), stop=(ch >= NCHUNK - 4),
                            tile_position=(0, 32 * j),
                            skip_group_check=True,
                        )
                    pend = []
                if weave and pr in (2, 6, 10, 14):
                    subset_chunk(weave.pop(0))
            dr = dpool.tile([128, 512], f32, name="dr")
            nc.vector.tensor_copy(dr, accT)
            for j in range(4):
                nc.sync.dma_start(
                    out=out_d[8 * j:8 * j + 8, t * 512:(t + 1) * 512],
                    in_=dr[32 * j:32 * j + 8, :])

        for wi, warm in enumerate((s_subp, s_xs, s_id)):
            warm_fence(wi, warm)
        for qc in range(4):
            subset_chunk(qc, prologue=True)
        for wi, warm in enumerate((s_xa, s_patm, s_pw)):
            warm_fence(3 + wi, warm)
        for t in range(NT):
            weave = list(range(4 * t + 4, 4 * t + 8)) if t < NT - 1 else []
            main_tile(t, weave)
            if t == NT - 2:
                # row 0 is fully written once tile 6's weave ran; export it
                # while tile 7 computes so it is off the tail
                nc.sync.dma_start(out=srow_d, in_=s_xa[0:1, :])

    # This walrus build permits exactly ONE sync wait per instruction
    # (setupSyncWait raises "Too many sync wait commands" at 2). Tile emits
    # several on converging dependencies; move extras onto preceding
    # same-engine NoOps, each carrying a single wait.
    for blk in nc.m.functions[0].blocks:
        out, changed = [], False
        for ins in blk.instructions:
            si = ins.sync_info
            w = list(si.on_wait) if si is not None and si.on_wait else []
            if len(w) > 1:
                for k, extra in enumerate(w[:-1]):
                    out.append(mybir.InstNoOp(
                        name=f"{ins.name}_sw{k}", engine=ins.engine,
                        sync_info=mybir.SyncInfo(on_wait=[extra], on_update=[]),
                        bass_nofuse=True))
                ins.sync_info = mybir.SyncInfo(
                    on_wait=[w[-1]], on_update=list(si.on_update))
                changed = True
            out.append(ins)
        if changed:
            blk.instructions = out

    _prog_cache["nc"] = nc
    return nc


def _im2col(arr, wrap):
    # [M,C,H,W] -> [M*H*W, C*9]; row m*H*W + h*W + w; col c*9 + di*3 + dj
    if wrap:
        ap = np.pad(arr, ((0, 0), (0, 0), (1, 1), (1, 1)), mode="wrap")
    else:
        ap = np.pad(arr, ((0, 0), (0, 0), (1, 1), (1, 1)))
    sl = [ap[:, :, di:di + H, dj:dj + W] for di in range(3) for dj in range(3)]
    st = np.stack(sl, axis=2)  # [M,C,9,H,W]
    return st.transpose(0, 3, 4, 1, 2).reshape(arr.shape[0] * H * W, C * 9)


def _split(v):
    # fp32 -> (hi, lo) bf16 pair with v ~= hi + lo to ~2^-16 relative
    h = v.astype(BF16)
    l = (v - h.astype(np.float32)).astype(BF16)
    return h, l


def _run(inputs, trace=False):
    from concourse.bass_utils import run_bass_kernel_spmd

    x = np.ascontiguousarray(np.asarray(inputs["x"], dtype=np.float32))
    images = np.ascontiguousarray(np.asarray(inputs["images"], dtype=np.float32))
    t = int(np.asarray(inputs["t"]))
    mu = float(np.asarray(inputs["mu_sched"])[t])
    sigma = float(np.asarray(inputs["sigma_sched"])[t])
    a = mu / (sigma * sigma)
    inv2s2 = 1.0 / (2.0 * sigma * sigma)

    xq = _im2col(x, wrap=True)                  # [NQ, 27]
    patches = _im2col(images, wrap=False)       # [NP, 27]
    c_all = ((-mu * mu * inv2s2) * np.sum(patches * patches, axis=1)).astype(np.float32)
    pcent = patches.reshape(NP, C, 9)[:, :, 4]  # [NP, 3]

    # x-side rows pre-scaled by A' so PSUM = A'*(g - s) + B'' directly
    xh, xl = _split((AP_SCALE * np.float32(a) * xq).astype(np.float32))
    xhu, _ = _split((a * xq).astype(np.float32))   # unscaled, subset pass

    xa = np.zeros((KA, NQ), BF16)
    xa[1:28] = xh.T
    xa[28:55] = xh.T
    xa[55:82] = xl.T
    xa[82:109] = xl.T
    xa[109] = BF16(1.0)
    xa[110] = BF16(1.0)

    xs = np.zeros((32, NQ), BF16)               # subset-pass lhsT (hi only)
    xs[0:27] = xhu.T
    xs[27] = BF16(1.0)

    # -A' * identity: the shift transpose directly scales gsub
    ident = (-float(AP_SCALE) * np.eye(128, dtype=np.float32)).astype(BF16)

    in_maps = []
    for cc in range(NCORES):
        lo = cc * PLOC
        ph, pl = _split(patches[lo:lo + PLOC])
        ch, cl = _split((AP_SCALE * c_all[lo:lo + PLOC]).astype(np.float32))
        patm = np.zeros((KA, PLOC), BF16)
        patm[0] = BF16(1.0)
        patm[1:28] = ph.T
        patm[28:55] = pl.T
        patm[55:82] = ph.T
        patm[82:109] = pl.T
        patm[109] = ch
        patm[110] = cl
        subp = np.zeros((32, NSUB), BF16)
        subp[0:27] = ph.T[:, ::SUB_STRIDE]
        subp[27] = c_all[lo:lo + PLOC][::SUB_STRIDE].astype(BF16)
        pwh, pwl = _split(pcent[lo:lo + PLOC])
        pw = np.zeros((128, 256), BF16)
        for chnk in range(NCHUNK):
            pw[:, chnk * 8:chnk * 8 + 3] = pwh[chnk * 128:(chnk + 1) * 128]
            pw[:, chnk * 8 + 3:chnk * 8 + 6] = pwl[chnk * 128:(chnk + 1) * 128]
            pw[:, chnk * 8 + 6] = BF16(1.0)
        in_maps.append({
            "patm": patm, "xa": xa, "xs": xs, "subp": subp, "pw": pw,
            "ident": ident,
        })

    nc = _build_program()
    res = run_bass_kernel_spmd(nc, in_maps, core_ids=list(range(NCORES)),
                               trace=trace)

    # host merge: partials are scaled by e^{-s_eff_c}; rescale to common max
    # s_eff = (B'' - v)/A' recovered exactly from the exported bf16 row
    v = np.stack([r["srow"][0].astype(np.float64) for r in res.results])
    s = (np.float64(BPP) - v) / np.float64(AP_SCALE)          # [8, NQ]
    part = np.stack([r["out"] for r in res.results])          # [8, 32, NQ]
    part = part.reshape(NCORES, 4, 8, NQ).sum(axis=1)         # sum col-groups
    S = s.max(axis=0)
    fac = np.exp(s - S[None, :])                              # [8, NQ] <= 1
    sum_w = (part[:, 6].astype(np.float64) * fac).sum(axis=0)              # [NQ]
    wc = ((part[:, 0:3] + part[:, 3:6]).astype(np.float64)
          * fac[:, None, :]).sum(axis=0)                      # [3, NQ]

    xcT = x.reshape(B, C, H * W).transpose(1, 0, 2).reshape(C, NQ)
    out_q = (mu * wc / sum_w[None, :] - xcT) / (sigma * sigma)  # [3, NQ]
    out = out_q.reshape(C, B, H, W).transpose(1, 0, 2, 3).astype(np.float32)
    return out, res


def kernel(**inputs) -> np.ndarray:
    out, _ = _run(inputs, trace=False)
    return out


# revision 19
# speedup vs baseline: 1.2471x; 1.0253x over previous
"""Trainium2 Bass kernel for nn_EquivariantLocalScoreMachine.

Math: for each query pixel q (B*H*W=4096, 27-dim circular 3x3 patch xq) over
dataset patches p (N*H*W=32768, zero-padded 3x3 patches):
    log_w[q,p] = -(||xq - mu*patch_p||^2) / (2 sigma^2)
               = b[q] + a*<xq, patch_p> + c[p],   a = mu/sigma^2
The per-q term b[q] cancels in the final ratio and is dropped.  Output:
    out[q,ch] = (mu * wc[q,ch]/sum_w[q] - x[q,ch]) / sigma^2
with softmax-style weights over p.

Sharding: patch axis split across 8 cores (4096 each); each core computes
partial (sum_w, wc) for all queries under a per-q shift; host merges with an
exact logsumexp rescale in fp64.

v2 design: the score matmul directly produces u = A'*(g - s) + B'' in PSUM,
where A' = 128*log2(e) and B'' = 16256 - 5.513.  In this scale, u IS the
bf16 bit pattern of ~exp(g-s) (Schraudolph).  The exp work is then split
across two engines:
  - ScalarE chunks: activation Exp with free affine scale=1/A', bias=-B''/A'
    (exact exp, bf16 out).
  - DVE chunks: one stock tensor_scalar int16(max(u,0)) whose int16 output
    bits, reinterpreted as bf16, are exp(g-s)*(1 +- 3%).  z < -88 clamps to
    +0.0; round-to-nearest int convert verified on HW.
Combined exp throughput 1.2+0.96 = 2.16 G elem/s/lane vs 1.2 ScalarE-only.

The weighted-center accumulation uses 4 concurrent col-group matmuls
(tile_position=(0,32j), M=8 each) so 4 p-chunks accumulate per 512-col
stream; partial groups are summed on the host.

Per core: subset pass (stride-16 patch subset, bf16 matmul + DVE reduce_max)
estimates the per-q max; the shift row v = -A'*(g_sub + MARGIN) + B'' is
transposed into xa row 0 via a -A'-scaled identity matmul and exported so the
host knows the exact shift s_eff = (B'' - v)/A'.

Every TPB instruction in this walrus build may carry at most ONE sync wait:
tiny PE "fence" matmuls pre-absorb cross-engine semaphores on hot paths, and
a post-scheduling pass splits any remaining multi-wait instruction into
single-wait NoOps.
"""
import sys
import numpy as np

for _p in ("/opt/trn_rl_repo", "/opt/pypackages"):
    if _p not in sys.path:
        sys.path.append(_p)

import ml_dtypes

BF16 = ml_dtypes.bfloat16

B, C, H, W = 4, 3, 32, 32
N_IMG = 32
NQ = B * H * W            # 4096 queries
NP = N_IMG * H * W        # 32768 dataset patches
NCORES = 8
PLOC = NP // NCORES       # 4096 patches per core
NCHUNK = PLOC // 128      # 32 p-chunks per core
NQC = NQ // 128           # 32 q-chunks (subset pass)
NT = NQ // 512            # 8 q-tiles (main pass)
FD = 1024                 # A-tile free dim (2 chunks per exp call)
SUB_STRIDE = 16
NSUB = PLOC // SUB_STRIDE  # 256 subset patches per core (max gap 95 < 128)
MARGIN = 40.0
KA = 111                  # packed contraction: 4*27 + c_hi + c_lo + shift

AP_SCALE = np.float32(184.66266)      # A' = 128*log2(e)
BPP = np.float32(16256.0 - 5.513)     # B'': bf16 bias minus Schraudolph center
K0 = np.float32(BPP - AP_SCALE * np.float32(MARGIN))
EXP_SCALE = float(1.0 / np.float64(AP_SCALE))
EXP_BIAS = float(-np.float64(BPP) / np.float64(AP_SCALE))
# pr-groups (2 chunks each) handled by the DVE Schraudolph path; rest ScalarE
DVE_GROUPS = frozenset((1, 3, 5, 7, 9, 11))

_prog_cache = {}


def _build_program():
    if "nc" in _prog_cache:
        return _prog_cache["nc"]
    from contextlib import ExitStack
    import concourse.bass as bass
    import concourse.tile as tile
    from concourse import mybir

    f32 = mybir.dt.float32
    bf = mybir.dt.bfloat16
    i16 = mybir.dt.int16
    nc = bass.Bass("TRN2", num_devices=NCORES, debug=False)
    patm_d = nc.dram_tensor("patm", [KA, PLOC], bf, kind="ExternalInput").ap()
    xa_d = nc.dram_tensor("xa", [KA, NQ], bf, kind="ExternalInput").ap()
    xs_d = nc.dram_tensor("xs", [32, NQ], bf, kind="ExternalInput").ap()
    subp_d = nc.dram_tensor("subp", [32, NSUB], bf, kind="ExternalInput").ap()
    pw_d = nc.dram_tensor("pw", [128, 256], bf, kind="ExternalInput").ap()
    ident_d = nc.dram_tensor("ident", [128, 128], bf, kind="ExternalInput").ap()
    out_d = nc.dram_tensor("out", [32, NQ], f32, kind="ExternalOutput").ap()
    srow_d = nc.dram_tensor("srow", [1, NQ], bf, kind="ExternalOutput").ap()

    with tile.TileContext(nc) as tc, ExitStack() as ctx:
        consts = ctx.enter_context(tc.tile_pool(name="consts", bufs=1))
        # PSUM (8 banks): psA 3x[128,1024]=6, scratch bank 1 (subset scores +
        # fence junk + transposed shift row), acc 1x[128,512]=1
        ps_big = ctx.enter_context(tc.tile_pool(name="ps_big", bufs=3, space="PSUM"))
        ps_fence = ctx.enter_context(
            tc.tile_pool(name="ps_fence", bufs=1, space="PSUM"))
        ps_acc = ctx.enter_context(tc.tile_pool(name="ps_acc", bufs=1, space="PSUM"))
        wpool = ctx.enter_context(tc.tile_pool(name="wpool", bufs=6))
        spool = ctx.enter_context(tc.tile_pool(name="spool", bufs=2))
        dpool = ctx.enter_context(tc.tile_pool(name="dpool", bufs=2))

        # All sync DMAs serialize on one HWDGE queue (~100GB/s), so order by
        # first use. xa row 0 is memset on-device (the shift writes replace
        # it) so the shift path never waits on the 0.9MB bulk transfer; patm
        # is split so the first p-chunks arrive before the main loop needs
        # them.
        s_subp = consts.tile([32, NSUB], bf)
        nc.sync.dma_start(out=s_subp, in_=subp_d)
        s_xs = consts.tile([32, NQ], bf)
        nc.sync.dma_start(out=s_xs, in_=xs_d)
        s_id = consts.tile([128, 128], bf)
        nc.sync.dma_start(out=s_id, in_=ident_d)
        # xa row 0 is NOT transferred or memset: every column is written by a
        # subset-chunk shift add before any reader touches it.
        s_xa = consts.tile([KA, NQ], bf)
        nc.sync.dma_start(out=s_xa[1:KA, 0:512], in_=xa_d[1:KA, 0:512])
        s_patm = consts.tile([KA, PLOC], bf)
        nc.sync.dma_start(out=s_patm[:, 0:1024], in_=patm_d[:, 0:1024])
        s_pw = consts.tile([128, 256], bf)
        nc.sync.dma_start(out=s_pw, in_=pw_d)
        nc.sync.dma_start(out=s_patm[:, 1024:2048], in_=patm_d[:, 1024:2048])
        nc.sync.dma_start(out=s_patm[:, 2048:3072], in_=patm_d[:, 2048:3072])
        nc.sync.dma_start(out=s_patm[:, 3072:], in_=patm_d[:, 3072:])
        nc.sync.dma_start(out=s_xa[1:KA, 512:], in_=xa_d[1:KA, 512:])

        # per-partition bias column for the ScalarE exp affine
        s_bias = consts.tile([128, 1], f32)
        nc.vector.memset(s_bias, EXP_BIAS)

        # wfence: long-lived scratch bank. Fence matmuls write junk columns
        # [0:16); [128:256) holds each q-chunk's transposed shift row briefly;
        # [256:256+NSUB) is the subset score region.
        wfence = ps_fence.tile([128, 512], f32, name="wfence")

        def warm_fence(wi, warm):
            nc.tensor.matmul(wfence[0:1, wi:wi + 1], warm[0:32, 0:1],
                             warm[0:32, 0:1], start=True, stop=True)

        # Subset-max work for one q-chunk: bf16 scores over the patch subset,
        # DVE reduce_max, then v = -A'*gsub + K0 transposed into xa row 0.
        gsubs = {}

        def subset_chunk(qc, prologue=False):
            if qc >= 1:
                # absorb the DVE tick of reduce(qc-1), whose scratch region
                # (or pool slot) the matmuls below overwrite
                nc.tensor.matmul(wfence[0:1, 8 + (qc % 2):9 + (qc % 2)],
                                 gsubs[qc - 1], s_id[0:128, 0:1],
                                 start=True, stop=True)
            if prologue:
                ps_sub = ps_big.tile([128, NSUB], f32, tag="big",
                                     name="ps_sub")
            else:
                ps_sub = wfence[:, 256:256 + NSUB]
            for h in range((NSUB + 511) // 512):
                n0, n1 = h * 512, min((h + 1) * 512, NSUB)
                nc.tensor.matmul(
                    ps_sub[:, n0:n1],
                    s_xs[:, qc * 128:(qc + 1) * 128],
                    s_subp[:, n0:n1],
                    start=True, stop=True,
                )
            # bf16 gsub (error < +-2, absorbed by MARGIN; exact shift is
            # exported via srow so no output error)
            gsub = spool.tile([128, 1], bf, name="gsub")
            gsubs[qc] = gsub
            nc.vector.reduce_max(out=gsub, in_=ps_sub, axis=mybir.AxisListType.X)
            # rowp = -A' * gsub  (s_id = -A'*I)
            rowp = wfence[0:1, 128:256]
            nc.tensor.matmul(rowp, gsub, s_id, start=True, stop=True)
            # xa row 0 <- rowp + K0, converted to bf16 on write
            nc.vector.tensor_scalar_add(
                s_xa[0:1, qc * 128:(qc + 1) * 128], rowp, float(K0))

        # Main work for one q-tile: u-scores -> exp split across ScalarE
        # (exact, affine-folded) and DVE (Schraudolph int16 bit trick) ->
        # [wc_hi|wc_lo|sum_w] accumulated in 4 concurrent PSUM col-groups.
        def main_tile(t, weave):
            # absorb the DVE ticks of this q-tile's row-0 shift writes
            nc.tensor.matmul(wfence[0:1, 0:512],
                             s_xa[0:1, t * 512:t * 512 + 1],
                             s_xa[0:1, t * 512:(t + 1) * 512],
                             start=True, stop=True)
            accT = ps_acc.tile([128, 512], f32, tag="acc", name="accT")
            # acc matmuls are deferred one pr-group and emitted in batches of
            # four CONSECUTIVE chunks (4 distinct col-groups): by batch time
            # only the newest wgt still gates, so the PE runs all four as one
            # concurrent col-tiled burst (~1 stream instead of 4).
            pend = []
            for pr in range(NCHUNK // 2):
                psA = ps_big.tile([128, FD], f32, tag="big", name="psA")
                for i in range(2):
                    ch = pr * 2 + i
                    nc.tensor.matmul(
                        psA[:, i * 512:(i + 1) * 512],
                        s_patm[:, ch * 128:(ch + 1) * 128],
                        s_xa[:, t * 512:(t + 1) * 512],
                        start=True, stop=True,
                    )
                wgt = wpool.tile([128, FD], bf, name="wgt")
                if pr in DVE_GROUPS:
                    nc.vector.tensor_scalar_max(wgt.bitcast(i16), psA, 0.0)
                else:
                    nc.scalar.activation(wgt, psA,
                                         mybir.ActivationFunctionType.Exp,
                                         bias=s_bias, scale=EXP_SCALE)
                for i in range(2):
                    pend.append((pr * 2 + i, wgt, i))
                if pr % 2 == 1:
                    for ch, w, i in pend:
                        j = ch % 4
                        nc.tensor.matmul(
                            accT[32 * j:32 * j + 8, :],
                            s_pw[:, ch * 8:(ch + 1) * 8],
                            w[:, i * 512:(i + 1) * 512],
                            start=(ch < 4), stop=(ch >= NCHUNK - 4),
                            tile_position=(0, 32 * j),
                            skip_group_check=True,
                        )
                    pend = []
                if weave and pr in (2, 6, 10, 14):
                    subset_chunk(weave.pop(0))
            dr = dpool.tile([128, 512], f32, name="dr")
            nc.vector.tensor_copy(dr, accT)
            for j in range(4):
                nc.sync.dma_start(
                    out=out_d[8 * j:8 * j + 8, t * 512:(t + 1) * 512],
                    in_=dr[32 * j:32 * j + 8, :])

        for wi, warm in enumerate((s_subp, s_xs, s_id)):
            warm_fence(wi, warm)
        for qc in range(4):
            subset_chunk(qc, prologue=True)
        for wi, warm in enumerate((s_xa, s_patm, s_pw)):
            warm_fence(3 + wi, warm)
        for t in range(NT):
            weave = list(range(4 * t + 4, 4 * t + 8)) if t < NT - 1 else []
            main_tile(t, weave)
            if t == NT - 2:
                # row 0 is fully written once tile 6's weave ran; export it
                # while tile 7 computes so it is off the tail
                nc.sync.dma_start(out=srow_d, in_=s_xa[0:1, :])

    # This walrus build permits exactly ONE sync wait per instruction
    # (setupSyncWait raises "Too many sync wait commands" at 2). Tile emits
    # several on converging dependencies; move extras onto preceding
    # same-engine NoOps, each carrying a single wait.
    for blk in nc.m.functions[0].blocks:
        out, changed = [], False
        for ins in blk.instructions:
            si = ins.sync_info
            w = list(si.on_wait) if si is not None and si.on_wait else []
            if len(w) > 1:
                for k, extra in enumerate(w[:-1]):
                    out.append(mybir.InstNoOp(
                        name=f"{ins.name}_sw{k}", engine=ins.engine,
                        sync_info=mybir.SyncInfo(on_wait=[extra], on_update=[]),
                        bass_nofuse=True))
                ins.sync_info = mybir.SyncInfo(
                    on_wait=[w[-1]], on_update=list(si.on_update))
                changed = True
            out.append(ins)
        if changed:
            blk.instructions = out

    _prog_cache["nc"] = nc
    return nc


def _im2col(arr, wrap):
    # [M,C,H,W] -> [M*H*W, C*9]; row m*H*W + h*W + w; col c*9 + di*3 + dj
    if wrap:
        ap = np.pad(arr, ((0, 0), (0, 0), (1, 1), (1, 1)), mode="wrap")
    else:
        ap = np.pad(arr, ((0, 0), (0, 0), (1, 1), (1, 1)))
    sl = [ap[:, :, di:di + H, dj:dj + W] for di in range(3) for dj in range(3)]
    st = np.stack(sl, axis=2)  # [M,C,9,H,W]
    return st.transpose(0, 3, 4, 1, 2).reshape(arr.shape[0] * H * W, C * 9)


def _split(v):
    # fp32 -> (hi, lo) bf16 pair with v ~= hi + lo to ~2^-16 relative
    h = v.astype(BF16)
    l = (v - h.astype(np.float32)).astype(BF16)
    return h, l


def _run(inputs, trace=False):
    from concourse.bass_utils import run_bass_kernel_spmd

    x = np.ascontiguousarray(np.asarray(inputs["x"], dtype=np.float32))
    images = np.ascontiguousarray(np.asarray(inputs["images"], dtype=np.float32))
    t = int(np.asarray(inputs["t"]))
    mu = float(np.asarray(inputs["mu_sched"])[t])
    sigma = float(np.asarray(inputs["sigma_sched"])[t])
    a = mu / (sigma * sigma)
    inv2s2 = 1.0 / (2.0 * sigma * sigma)

    xq = _im2col(x, wrap=True)                  # [NQ, 27]
    patches = _im2col(images, wrap=False)       # [NP, 27]
    c_all = ((-mu * mu * inv2s2) * np.sum(patches * patches, axis=1)).astype(np.float32)
    pcent = patches.reshape(NP, C, 9)[:, :, 4]  # [NP, 3]

    # x-side rows pre-scaled by A' so PSUM = A'*(g - s) + B'' directly
    xh, xl = _split((AP_SCALE * np.float32(a) * xq).astype(np.float32))
    xhu, _ = _split((a * xq).astype(np.float32))   # unscaled, subset pass

    xa = np.zeros((KA, NQ), BF16)
    xa[1:28] = xh.T
    xa[28:55] = xh.T
    xa[55:82] = xl.T
    xa[82:109] = xl.T
    xa[109] = BF16(1.0)
    xa[110] = BF16(1.0)

    xs = np.zeros((32, NQ), BF16)               # subset-pass lhsT (hi only)
    xs[0:27] = xhu.T
    xs[27] = BF16(1.0)

    # -A' * identity: the shift transpose directly scales gsub
    ident = (-float(AP_SCALE) * np.eye(128, dtype=np.float32)).astype(BF16)

    in_maps = []
    for cc in range(NCORES):
        lo = cc * PLOC
        ph, pl = _split(patches[lo:lo + PLOC])
        ch, cl = _split((AP_SCALE * c_all[lo:lo + PLOC]).astype(np.float32))
        patm = np.zeros((KA, PLOC), BF16)
        patm[0] = BF16(1.0)
        patm[1:28] = ph.T
        patm[28:55] = pl.T
        patm[55:82] = ph.T
        patm[82:109] = pl.T
        patm[109] = ch
        patm[110] = cl
        subp = np.zeros((32, NSUB), BF16)
        subp[0:27] = ph.T[:, ::SUB_STRIDE]
        subp[27] = c_all[lo:lo + PLOC][::SUB_STRIDE].astype(BF16)
        pwh, pwl = _split(pcent[lo:lo + PLOC])
        pw = np.zeros((128, 256), BF16)
        for chnk in range(NCHUNK):
            pw[:, chnk * 8:chnk * 8 + 3] = pwh[chnk * 128:(chnk + 1) * 128]
            pw[:, chnk * 8 + 3:chnk * 8 + 6] = pwl[chnk * 128:(chnk + 1) * 128]
            pw[:, chnk * 8 + 6] = BF16(1.0)
        in_maps.append({
            "patm": patm, "xa": xa, "xs": xs, "subp": subp, "pw": pw,
            "ident": ident,
        })

    nc = _build_program()
    res = run_bass_kernel_spmd(nc, in_maps, core_ids=list(range(NCORES)),
                               trace=trace)

    # host merge: partials are scaled by e^{-s_eff_c}; rescale to common max
    # s_eff = (B'' - v)/A' recovered exactly from the exported bf16 row
    v = np.stack([r["srow"][0].astype(np.float64) for r in res.results])
    s = (np.float64(BPP) - v) / np.float64(AP_SCALE)          # [8, NQ]
    part = np.stack([r["out"] for r in res.results])          # [8, 32, NQ]
    part = part.reshape(NCORES, 4, 8, NQ).sum(axis=1)         # sum col-groups
    S = s.max(axis=0)
    fac = np.exp(s - S[None, :])                              # [8, NQ] <= 1
    sum_w = (part[:, 6].astype(np.float64) * fac).sum(axis=0)              # [NQ]
    wc = ((part[:, 0:3] + part[:, 3:6]).astype(np.float64)
          * fac[:, None, :]).sum(axis=0)                      # [3, NQ]

    xcT = x.reshape(B, C, H * W).transpose(1, 0, 2).reshape(C, NQ)
    out_q = (mu * wc / sum_w[None, :] - xcT) / (sigma * sigma)  # [3, NQ]
    out = out_q.reshape(C, B, H, W).transpose(1, 0, 2, 3).astype(np.float32)
    return out, res


def kernel(**inputs) -> np.ndarray:
    out, _ = _run(inputs, trace=False)
    return out
